# revision 1
# baseline (speedup 1.0000x reference)
# GATConv Trainium kernel: host prep + Bass program builder (parameterized).
import numpy as np
import ml_dtypes
import concourse.bass as bass
import concourse.bacc as bacc
import concourse.mybir as mybir
import concourse.tile as tile
from concourse._compat import exact_div

F32 = mybir.dt.float32
BF16 = mybir.dt.bfloat16
I16 = mybir.dt.int16

ALPHA = 0.2
H, D = 8, 32
HD = H * D            # 256
IN = 256
ELER_W = 64           # padded er row (f32) -> 256B stride
FT_W = 384            # padded ft row (bf16) -> 768B stride; cols 0:256 ft, 256:272 el(f32 bytes)
WMAX = 8              # max blocks (128 idx each) per gather window (1024-desc ring)


def _ceil(a, b):
    return -(-a // b)


class Plan:
    """Host-side uniform schedule shared by all cores."""

    def __init__(self, N, E, src, dst, n_cores, tiles_per_core, st_tiles=6, chunk=32768):
        self.N, self.E, self.C = N, E, n_cores
        self.NT = tiles_per_core              # dst tiles per core
        self.ND = tiles_per_core * 128        # dsts per core
        self.NN = self.ND * n_cores           # padded node count (tables)
        assert self.NN >= N
        self.chunk = chunk
        self.NQ = _ceil(self.NN, chunk)       # chunks
        self.ST = st_tiles

        order = np.argsort(dst, kind="stable")
        src_s, dst_s = src[order], dst[order]
        core_of = dst_s // self.ND
        tile_of = (dst_s % self.ND) // 128
        # rotated src coordinate per edge: (src - core*ND) mod NN
        rot_src = (src_s - core_of * self.ND) % self.NN
        q_of = rot_src // chunk
        self._rot_src = rot_src

        cnt = np.zeros((n_cores, self.NT, self.NQ), dtype=np.int64)
        np.add.at(cnt, (core_of, tile_of, q_of), 1)
        B = _ceil(cnt, 128).max(axis=0)       # [NT, NQ] blocks per group
        B[:, 0] = np.maximum(B[:, 0], 1)      # every tile has >=1 block
        self.B = B
        self.blocks_per_tile = B.sum(axis=1)  # [NT]

        # per-(c,t,q) edge lists
        self.edges = [[[None] * self.NQ for _ in range(self.NT)] for _ in range(n_cores)]
        key = ((core_of * self.NT + tile_of) * self.NQ + q_of)
        order2 = np.argsort(key, kind="stable")
        ks = key[order2]
        bounds = np.searchsorted(ks, np.arange(n_cores * self.NT * self.NQ + 1))
        for c in range(n_cores):
            for t in range(self.NT):
                for q in range(self.NQ):
                    k = (c * self.NT + t) * self.NQ + q
                    sel = order2[bounds[k]:bounds[k + 1]]
                    self.edges[c][t][q] = (rot_src[sel], dst_s[sel])

        # call schedule: supertile-major, chunk-minor
        self.n_st = _ceil(self.NT, st_tiles)
        self.calls = []                       # (q, [(t, B_tq), ...])
        for s in range(self.n_st):
            ts = range(s * st_tiles, min((s + 1) * st_tiles, self.NT))
            for q in range(self.NQ):
                items = [(t, int(B[t, q])) for t in ts if B[t, q] > 0]
                if items:
                    self.calls.append((q, items))
        self.NBtot = int(B.sum() * 128)

    def build_streams(self, c):
        NB = self.NBtot
        idx_ft = np.zeros(NB, dtype=np.int16)
        idx_er = np.zeros(NB, dtype=np.int16)
        dstl = np.full(NB, 200.0, dtype=np.float32)
        pos = 0
        for q, items in self.calls:
            for t, nb in items:
                s_arr, d_arr = self.edges[c][t][q]
                n = len(s_arr)
                assert n <= nb * 128
                idx_ft[pos:pos + n] = (s_arr - q * self.chunk).astype(np.int16)
                assert (s_arr >= q * self.chunk).all() and (s_arr < (q + 1) * self.chunk).all()
                idx_er[pos:pos + n] = (d_arr - c * self.ND).astype(np.int16)
                dstl[pos:pos + n] = (d_arr - (c * self.ND + t * 128)).astype(np.float32)
                pos += nb * 128
        assert pos == NB

        nb16 = NB // 16
        ift = np.zeros((128, nb16), dtype=np.int16)
        ier = np.zeros((128, nb16), dtype=np.int16)
        i = np.arange(NB)
        for k in range(8):
            ift[16 * k + i % 16, i // 16] = idx_ft
            ier[16 * k + i % 16, i // 16] = idx_er
        dl = np.zeros((128, NB // 128), dtype=np.float32)
        dl[i % 128, i // 128] = dstl
        dlb = np.broadcast_to(dstl.astype(ml_dtypes.bfloat16), (128, NB))
        return {"idx_ft": ift, "idx_er": ier, "dstl": dl,
                "dstl_bcast": np.ascontiguousarray(dlb)}


def make_waug(W, attn_l, attn_r):
    """[IN, 272] f32 cols: [W'^T | Ml | Mr]; W' rows in d-major order d*H+h."""
    perm = np.empty(HD, dtype=np.int64)
    for h in range(H):
        for d in range(D):
            perm[d * H + h] = h * D + d
    Wp = W[perm, :]                                   # [256, IN]
    Ml = np.zeros((IN, H), dtype=np.float32)
    Mr = np.zeros((IN, H), dtype=np.float32)
    for h in range(H):
        rows = W[h * D:(h + 1) * D, :]                # [D, IN]
        Ml[:, h] = attn_l[0, h, :] @ rows
        Mr[:, h] = attn_r[0, h, :] @ rows
    return np.concatenate([Wp.T, Ml, Mr], axis=1).astype(np.float32)


def dma_gather_raw(gp, out_ap, in_ap, idxs_ap, num_idxs, elem_size, elem_step,
                   queue_num=0):
    """dma_gather minus the elem_size%256 assert (row stride must be %256B)."""
    stride_bytes = elem_step * mybir.dt.size(in_ap.dtype)
    stride_bytes_256 = exact_div(stride_bytes, 256)
    _in_ap = gp.lower_ap_dma(in_ap, for_custom_bir_dma=True)
    _idxs_ap = gp.lower_ap(idxs_ap)
    _out_ap = gp.lower_ap(out_ap)
    return gp.add_instruction(
        mybir.InstDMAGatherAnt(
            name=gp.bass.get_next_instruction_name(),
            ins=[*_in_ap, _idxs_ap, gp.lower_val_access(gp.to_reg(num_idxs))],
            outs=[_out_ap],
            transpose=False, num_idxs=num_idxs, elem_size=elem_size,
            stride_bytes_256=stride_bytes_256, gen_mode=0, single_packet=True,
            queue_num=queue_num, sbuf_tokens_per_rank=0, sbuf_free_dim_per_rank=0,
            sbuf_free_dim_pad_per_rank=0, sbuf_byte_offset=0,
        )
    )


def build_program(plan, n_cores, fc_mega=16, stages=5, nq=1):
    """One SPMD Bass program. Inputs: featT bf16 [IN,NN], waug bf16 [IN,272],
    idx_ft/idx_er i16, dstl f32. Output: out [ND, 256] f32 (h-major cols)."""
    p = plan
    NN, ND, NT = p.NN, p.ND, p.NT
    nc = bacc.Bacc("TRN2", target_bir_lowering=False, debug=False, num_devices=n_cores, num_swdge_queues=nq)

    featT_d = nc.dram_tensor("featT", [IN, NN], BF16, kind="ExternalInput").ap()
    waug_d = nc.dram_tensor("waug", [IN, HD + 16], BF16, kind="ExternalInput").ap()
    ift_d = nc.dram_tensor("idx_ft", [128, p.NBtot // 16], I16, kind="ExternalInput").ap()
    dstl_d = nc.dram_tensor("dstl", [128, p.NBtot // 128], F32, kind="ExternalInput").ap()
    ier_d = nc.dram_tensor("idx_er", [128, p.NBtot // 16], I16, kind="ExternalInput").ap()
    dlb_d = nc.dram_tensor("dstl_bcast", [128, p.NBtot], BF16, kind="ExternalInput").ap()
    iotac_d = nc.dram_tensor("iotac", [128, 1], F32, kind="ExternalInput").ap()
    iota_d = nc.dram_tensor("iota", [128, 128], BF16, kind="ExternalInput").ap()
    ft_t = nc.dram_tensor("ft_tab", [NN, FT_W], BF16, kind="Internal").ap()
    eler_t = nc.dram_tensor("eler_tab", [NN, ELER_W], F32, kind="Internal").ap()
    out_d = nc.dram_tensor("out", [ND, HD], F32, kind="ExternalOutput").ap()

    n_nt = NN // 128
    MG = fc_mega

    with tile.TileContext(nc) as tc:
        # ---------------- Phase A: FC over all nodes ----------------
        with tc.tile_pool(name="fca", bufs=2) as pool, \
             tc.tile_pool(name="fcc", bufs=1) as cpool, \
             tc.tile_pool(name="fcp", bufs=4, space="PSUM") as psp:
            wa = cpool.tile([128, 2, HD + 16], BF16)
            nc.sync.dma_start(wa[:], waug_d.rearrange("(k p) c -> p k c", p=128))
            for g0 in range(0, n_nt, MG):
                gn = min(MG, n_nt - g0)
                ftin = pool.tile([128, 2, MG * 128], BF16, tag="ftin")
                nc.sync.dma_start(
                    ftin[:, :, :gn * 128],
                    featT_d.rearrange("(k p) n -> p k n", p=128)[:, :, g0 * 128:(g0 + gn) * 128])
                ftst = pool.tile([128, MG, HD + 16], BF16, tag="ftst")
                elst = pool.tile([128, MG, ELER_W], F32, tag="elst")
                nc.vector.memset(elst[:, :, 8:ELER_W], 0.0)
                for j in range(gn):
                    fc_ps = psp.tile([128, HD + 16], F32, tag="fc")
                    for k in range(2):
                        nc.tensor.matmul(fc_ps[:], ftin[:, k, j * 128:(j + 1) * 128],
                                         wa[:, k, :], start=(k == 0), stop=(k == 1))
                    nc.any.tensor_copy(ftst[:, j, 0:HD], fc_ps[:, 0:HD])
                    # el (f32) raw bytes into bf16 cols 256:272
                    nc.any.tensor_copy(ftst[:, j, HD:HD + 16].bitcast(F32),
                                       fc_ps[:, HD:HD + 8])
                    nc.any.tensor_copy(elst[:, j, 0:8], fc_ps[:, HD + 8:HD + 16])
                nc.sync.dma_start(
                    ft_t.rearrange("(g p) c -> p g c", p=128)[:, g0:g0 + gn, 0:HD + 16],
                    ftst[:, :gn, :])
                nc.sync.dma_start(
                    eler_t.rearrange("(g p) c -> p g c", p=128)[:, g0:g0 + gn, :],
                    elst[:, :gn, :])

        # ---------------- Phase B: edge pipeline ----------------
        with tc.tile_pool(name="ebc", bufs=1) as cpool, \
             tc.tile_pool(name="eb", bufs=3) as pool, \
             tc.tile_pool(name="oh", bufs=4) as ohpool, \
             tc.tile_pool(name="ebo", bufs=2) as opool, \
             tc.tile_pool(name="ebp", bufs=6, space="PSUM") as psp, \
             tc.tile_pool(name="ebx", bufs=2, space="PSUM") as psx:
            iota_row = cpool.tile([128, 128], BF16)
            nc.sync.dma_start(iota_row[:], iota_d[:])
            iotac = cpool.tile([128, 1], F32)
            nc.sync.dma_start(iotac[:], iotac_d[:])
            agg = {}
            issued = {t: 0 for t in range(NT)}
            ost = {}
            pos = 0
            wcount = 0
            cur_s = -1
            er_st = None
            calls = p.calls if stages >= 2 else []
            for q, items in calls:
                s_call = items[0][0] // p.ST
                if stages >= 3 and s_call != cur_s:
                    cur_s = s_call
                    t0s = s_call * p.ST
                    nst = min(p.ST, NT - t0s)
                    er_st = opool.tile([128, p.ST, ELER_W], F32,
                                       tag="erst", name=f"erst{s_call}")
                    nc.sync.dma_start(
                        er_st[:, :nst, :],
                        eler_t.rearrange("(g p) c -> p g c", p=128)[:, t0s:t0s + nst, :])
                # split the call's blocks into gather windows of <= WMAX blocks
                blocks = []                     # flat (t, j_of_t) per block
                for t, nbt in items:
                    blocks += [t] * nbt
                hi = min((q + 1) * p.chunk, NN)
                w0 = 0
                while w0 < len(blocks):
                    wn = min(WMAX, len(blocks) - w0)
                    NB = wn * 128
                    o16, o128 = pos // 16, pos // 128
                    ift = pool.tile([128, WMAX * 8], I16, tag="ift")
                    nc.sync.dma_start(ift[:, :NB // 16], ift_d[:, o16:o16 + NB // 16])
                    dstl = pool.tile([128, WMAX], F32, tag="dstl")
                    nc.sync.dma_start(dstl[:, :wn], dstl_d[:, o128:o128 + wn])
                    dlb = pool.tile([128, WMAX * 128], BF16, tag="dlb")
                    nc.sync.dma_start(dlb[:, :NB], dlb_d[:, pos:pos + NB])

                    g = pool.tile([128, WMAX, HD + 16], BF16, tag="g")
                    if stages >= 2:
                        dma_gather_raw(nc.gpsimd, g[:, :wn, :],
                                       ft_t[q * p.chunk:hi, 0:HD + 16],
                                       ift[:, :NB // 16], NB, HD + 16, FT_W,
                                       queue_num=wcount % nq)
                    lw = pool.tile([128, WMAX, H], F32, tag="lw")
                    if stages >= 3:
                        ohT = pool.tile([128, WMAX * 128], F32, tag="ohT")
                        nc.vector.tensor_scalar(ohT[:, :NB], dlb[:, :NB], iotac[:],
                                                None, mybir.AluOpType.is_equal)
                        erx_ps = psx.tile([128, WMAX * H], F32, tag="erx",
                                          name=f"erx{wcount}")
                        for j in range(wn):
                            t = blocks[w0 + j]
                            nc.tensor.matmul(erx_ps[:, j * H:(j + 1) * H],
                                             ohT[:, j * 128:(j + 1) * 128],
                                             er_st[:, t % p.ST, 0:H],
                                             start=True, stop=True,
                                             skip_group_check=True)
                        nc.vector.tensor_tensor(
                            lw[:, :wn, :], g[:, :wn, HD:HD + 16].bitcast(F32),
                            erx_ps.rearrange("p (b h) -> p b h", h=H)[:, :wn, :],
                            mybir.AluOpType.add)
                        nc.vector.scalar_tensor_tensor(lw[:, :wn, :], lw[:, :wn, :],
                                                       ALPHA, lw[:, :wn, :],
                                                       mybir.AluOpType.mult,
                                                       mybir.AluOpType.max)
                        ee = pool.tile([128, WMAX, H], F32, tag="ee")
                        nc.scalar.activation(ee[:, :wn, :], lw[:, :wn, :],
                                             mybir.ActivationFunctionType.Exp)
                    if stages >= 4:
                        rhs = pool.tile([128, WMAX, HD + 8], BF16, tag="rhs")
                        nc.vector.tensor_tensor(
                            rhs[:, :wn, 0:HD].rearrange("p b (d h) -> p b d h", h=H),
                            g[:, :wn, 0:HD].rearrange("p b (d h) -> p b d h", h=H),
                            ee[:, :wn, :].unsqueeze(2).broadcast_to([128, wn, D, H]),
                            mybir.AluOpType.mult)
                        nc.any.tensor_copy(rhs[:, :wn, HD:HD + 8], ee[:, :wn, :])
                    if stages >= 5:
                        for j in range(wn):
                            t = blocks[w0 + j]
                            if t not in agg:
                                agg[t] = psp.tile([128, HD + 8], F32, tag="agg",
                                                  name=f"agg{t}")
                            at = agg[t]
                            tot = int(p.blocks_per_tile[t])
                            oh = ohpool.tile([128, 128], BF16, tag="oh")
                            nc.vector.tensor_scalar(oh[:], iota_row[:],
                                                    dstl[:, j:j + 1], None,
                                                    mybir.AluOpType.is_equal)
                            nc.tensor.matmul(at[:], oh[:], rhs[:, j, :],
                                             start=(issued[t] == 0),
                                             stop=(issued[t] == tot - 1),
                                             skip_group_check=True)
                            issued[t] += 1
                            if issued[t] == tot:
                                s = t // p.ST
                                if s not in ost:
                                    ost[s] = opool.tile([128, p.ST, HD], F32,
                                                        tag="ost", name=f"ost{s}")
                                pool_ost = ost[s]
                                dsum = pool.tile([128, H], F32, tag="dsum")
                                nc.vector.tensor_scalar(dsum[:], at[:, HD:HD + 8],
                                                        1e-20, None,
                                                        mybir.AluOpType.max)
                                recd = pool.tile([128, H], F32, tag="recd")
                                nc.vector.reciprocal(recd[:], dsum[:])
                                nc.vector.tensor_tensor(
                                    pool_ost[:, t % p.ST, :].rearrange(
                                        "p (h d) -> p h d", d=D),
                                    at[:, 0:HD].rearrange("p (d h) -> p h d", h=H),
                                    recd[:].unsqueeze(2).broadcast_to([128, H, D]),
                                    mybir.AluOpType.mult)
                                del agg[t]
                                t0 = s * p.ST
                                n_in_st = min(p.ST, NT - t0)
                                if all(issued[tt] == int(p.blocks_per_tile[tt])
                                       for tt in range(t0, t0 + n_in_st)):
                                    nc.sync.dma_start(
                                        out_d.rearrange("(g p) c -> p g c", p=128)[:, t0:t0 + n_in_st, :],
                                        pool_ost[:, :n_in_st, :])
                                    del ost[s]
                    pos += NB
                    w0 += wn
                    wcount += 1
            assert pos == p.NBtot or stages < 2
    return _finish(nc)


def _finish(nc):
    nc.compile()
    return nc


def host_prep(feat, W, attn_l, attn_r, src, dst, n_cores, tiles_per_core,
              st_tiles=6, chunk=32768):
    N = feat.shape[0]
    E = src.shape[0]
    plan = Plan(N, E, src.astype(np.int64), dst.astype(np.int64), n_cores,
                tiles_per_core, st_tiles, chunk)
    featT = np.zeros((IN, plan.NN), dtype=ml_dtypes.bfloat16)
    featT[:, :N] = feat.T.astype(ml_dtypes.bfloat16)
    waug = make_waug(W, attn_l, attn_r).astype(ml_dtypes.bfloat16)
    in_maps = []
    for c in range(n_cores):
        s = plan.build_streams(c)
        rot = np.roll(featT, -c * plan.ND, axis=1)   # col j = node (c*ND+j) mod NN
        iota_np = np.broadcast_to(np.arange(128, dtype=np.float32),
                                  (128, 128)).astype(ml_dtypes.bfloat16)
        in_maps.append({
            "featT": np.ascontiguousarray(rot), "waug": waug,
            "idx_ft": s["idx_ft"], "idx_er": s["idx_er"], "dstl": s["dstl"],
            "dstl_bcast": s["dstl_bcast"],
            "iota": np.ascontiguousarray(iota_np),
            "iotac": np.arange(128, dtype=np.float32).reshape(128, 1),
        })
    return plan, in_maps


def assemble_output(plan, results, N):
    full = np.zeros((plan.NN, HD), dtype=np.float32)
    for c in range(plan.C):
        full[c * plan.ND:(c + 1) * plan.ND] = results[c]["out"]
    return full[:N].reshape(N, H, D)


# ----------------------------------------------------------------------------
# Harness entrypoint: full inputs in, full output out. Shapes hardcoded for
# nn_GATConv (N=100000, E=1600000, IN=256, H=8, D=32) on 8 NeuronCores.
# ----------------------------------------------------------------------------
from concourse.bass_interp import get_hw_module as _get_hw_module
from concourse import bass_utils as _bass_utils

_N_CORES = 8
_TPC = 98            # dst tiles per core (98*128*8 = 100352 >= 100000)
_ST_TILES = 4
_CHUNK = 32768
_NQ = 2              # SWDGE queues: g-gathers and er-gathers in parallel

_cache = {}


def kernel(feat, W, attn_l, attn_r, src, dst):
    feat = np.ascontiguousarray(np.asarray(feat, dtype=np.float32))
    W = np.ascontiguousarray(np.asarray(W, dtype=np.float32))
    attn_l = np.asarray(attn_l, dtype=np.float32)
    attn_r = np.asarray(attn_r, dtype=np.float32)
    src = np.asarray(src).astype(np.int64)
    dst = np.asarray(dst).astype(np.int64)
    N = feat.shape[0]

    plan, in_maps = host_prep(feat, W, attn_l, attn_r, src, dst,
                              _N_CORES, _TPC, st_tiles=_ST_TILES, chunk=_CHUNK)
    key = "prog"
    if key not in _cache:
        nc = build_program(plan, _N_CORES, nq=_NQ)
        nc.m = _get_hw_module(nc.m)
        _cache[key] = nc
    nc = _cache[key]
    res = _bass_utils.run_bass_kernel_spmd(nc, in_maps,
                                           core_ids=list(range(_N_CORES)))
    return assemble_output(plan, res.results, N)

